# revision 42
# baseline (speedup 1.0000x reference)
"""AddAttention (Bahdanau additive attention) Trainium2 kernel.

Math (per batch b):
    q   = query @ Wq + bq                          [D]
    k_t = value[t] @ Wk + bk                       [T, D]
    s_t = sum_d scale[d] * tanh(q[d] + k_t[d])     [T]
    a   = softmax(s masked to t < value_lens[b])
    out = sum_t a_t * value[t]                     [DV]

Distribution: pure data-parallel over batch B=32 across 8 NeuronCores
(4 batches per core, params replicated, no collectives).

Sparsity: rows t >= value_lens[b] are masked out of the softmax, so they
never influence the output.  Batches are sorted by value_lens and assigned
to (core, slot) so that slot s on every core holds a batch of similar
length; the compiled graph only processes ceil(max_len_in_slot/128) row
chunks per slot.  Masking (runtime data) handles the sub-chunk boundary,
so the same SPMD graph is correct on every core.

Layout/precision staging on the host (pure data movement + the same
f32->bf16 rounding the kernel would otherwise do on-chip): each core
receives its value rows twice in bf16 - natural [t, d] (feeds the final
context matmul) and transposed [d, t] (feeds the k = value @ Wk
projection, which contracts over d and therefore needs d on the
partition axis).  This removes all on-chip transposes and casts from the
critical path; every FLOP of the model still runs on the device.

Per-core pipeline (matmul compute bf16, accumulation fp32):
  - k projection: 4 accumulating matmuls per 128-row chunk with the
    transposed value tiles as the stationary operand, plus a 5th K=1
    matmul that broadcasts q (+bq +bk, computed on-chip) into the same
    PSUM accumulation - so ScalarE can apply tanh straight from PSUM
  - VectorE does a fused (tanh * scale) + free-axis-reduce into scores
  - softmax without max-subtraction (|s| <= sum|scale| ~ 20, safe in
    fp32); mask applied multiplicatively after exp; the normalization is
    folded into one per-batch scalar applied to the context row
  - context = attn @ value as M=1 accumulating matmuls over the natural
    bf16 value tiles; partition-sums for softmax run on the PE against a
    ones column
"""

import math
from contextlib import ExitStack

import ml_dtypes
import numpy as np

import concourse.bass as bass
import concourse.bacc as bacc
import concourse.tile as tile
from concourse import mybir
from concourse import bass_utils

F32 = mybir.dt.float32
BF16 = mybir.dt.bfloat16
I32 = mybir.dt.int32
AF = mybir.ActivationFunctionType
ALU = mybir.AluOpType

N_CORES = 8
B, TV, DQ, DV, D = 32, 2048, 512, 512, 512
SLOTS = B // N_CORES  # 4 batches per core
P = 128  # partitions / t-chunk rows
KC = D // P  # 4 contraction chunks of 128

BF16_NP = ml_dtypes.bfloat16


def build_graph(nchunks):
    """Build the per-core Bass graph. nchunks[s] = number of 128-row value
    chunks processed for slot s (same on every core -> same SPMD graph)."""
    nchunks = tuple(int(c) for c in nchunks)
    assert len(nchunks) == SLOTS and all(1 <= c <= TV // P for c in nchunks)
    nch_max = max(nchunks)
    rows = [P * c for c in nchunks]
    row_off = np.cumsum([0] + rows).tolist()
    R = row_off[-1]

    nc = bacc.Bacc("TRN2", target_bir_lowering=False, debug=False,
                   enable_asserts=False)

    value_d = nc.dram_tensor("value", [R, DV], BF16, kind="ExternalInput")
    valueT_d = nc.dram_tensor("valueT", [DV, R], BF16, kind="ExternalInput")
    query_d = nc.dram_tensor("queryT", [DQ, SLOTS], BF16, kind="ExternalInput")
    lens_d = nc.dram_tensor("value_lens", [SLOTS], I32, kind="ExternalInput")
    Wq_d = nc.dram_tensor("Wq", [DQ, D], BF16, kind="ExternalInput")
    bq_d = nc.dram_tensor("bq", [D], BF16, kind="ExternalInput")
    Wk_d = nc.dram_tensor("Wk", [DV, D], BF16, kind="ExternalInput")
    bk_d = nc.dram_tensor("bk", [D], BF16, kind="ExternalInput")
    scale_d = nc.dram_tensor("scale", [D], BF16, kind="ExternalInput")
    out_d = nc.dram_tensor("out", [SLOTS, DV], F32, kind="ExternalOutput")

    with tile.TileContext(nc) as tc, ExitStack() as ctx:
        consts = ctx.enter_context(tc.tile_pool(name="consts", bufs=1))
        vsb_pool = ctx.enter_context(tc.tile_pool(name="vsb", bufs=3))
        vt_pool = ctx.enter_context(tc.tile_pool(name="vt", bufs=3))
        th_pool = ctx.enter_context(tc.tile_pool(name="th", bufs=6))
        scrap_pool = ctx.enter_context(tc.tile_pool(name="scrap", bufs=4))
        sm_pool = ctx.enter_context(tc.tile_pool(name="sm", bufs=2))
        psum_pool = ctx.enter_context(
            tc.tile_pool(name="psum", bufs=8, space=bass.MemorySpace.PSUM))
        cps_pool = psum_pool

        # ---- constants / setup ----
        # all bf16 params arrive pre-cast; load them on the ScalarE HWDGE
        # queue so they don't contend with valueT (Sync) / value (GpSimd).
        # Order: Wk + the q-path inputs first - the PE instruction stream
        # is in-order and its head needs Wk (chunk matmuls) and q_flat
        # (folded into every chunk's accumulation group).
        # query transposed on host: QT[p, c, b] = queryT[c*128 + p, b]
        QT_sb = consts.tile([P, KC, SLOTS], BF16)
        nc.gpsimd.dma_start(
            QT_sb[:], query_d.ap().rearrange("(c p) b -> p c b", p=P))
        Wq_sb = consts.tile([P, KC, D], BF16)
        nc.gpsimd.dma_start(
            Wq_sb[:], Wq_d.ap().rearrange("(c p) n -> p c n", p=P))
        bq_row = consts.tile([1, D], BF16)
        nc.gpsimd.dma_start(bq_row[:], bq_d.ap().rearrange("(a d) -> a d", a=1))
        bk_row = consts.tile([1, D], BF16)
        nc.gpsimd.dma_start(bk_row[:], bk_d.ap().rearrange("(a d) -> a d", a=1))
        Wk_sb = consts.tile([P, KC, D], BF16)
        nc.scalar.dma_start(
            Wk_sb[:], Wk_d.ap().rearrange("(c p) n -> p c n", p=P))
        scale_row = consts.tile([1, D], BF16)
        nc.scalar.dma_start(
            scale_row[:], scale_d.ap().rearrange("(a d) -> a d", a=1))
        # broadcast scale across partitions (DMA replicated read: the
        # single-partition source is re-read P times via a 0-step free dim)
        scale_bc = consts.tile([P, D], BF16)
        nc.scalar.dma_start(scale_bc[:],
                            scale_row[:].unsqueeze(1).to_broadcast((1, P, D)))

        ones_row = consts.tile([1, SLOTS], BF16)
        nc.vector.memset(ones_row[:], 1.0)
        ones128 = consts.tile([1, P], BF16)
        nc.vector.memset(ones128[:], 1.0)
        ones_col_f = consts.tile([P, 1], F32)
        nc.vector.memset(ones_col_f[:], 1.0)
        ones512 = consts.tile([1, D], BF16)
        nc.vector.memset(ones512[:], 1.0)

        # PE warm-up: ~5 us of memset-only matmuls inside the startup DMA
        # window flips the HAM clock gate to 2.4 GHz before real work lands
        for w in range(12):
            wu = psum_pool.tile([P, D], F32, tag="kps", name=f"wu{w}")
            nc.tensor.matmul(wu[:], ones128[:], ones512[:],
                             start=True, stop=True)

        # value_lens -> f32 column, then per-slot broadcast column
        lens_i = consts.tile([SLOTS, 1], I32)
        nc.sync.dma_start(lens_i[:], lens_d.ap().rearrange("(b a) -> b a", a=1))
        lens_f = consts.tile([SLOTS, 1], F32)
        nc.vector.tensor_copy(lens_f[:], lens_i[:])

        # iota over t within the slot: iota[p, c] = c*128 + p
        iota_i = consts.tile([P, nch_max], I32)
        nc.gpsimd.iota(iota_i[:], pattern=[[P, nch_max]], base=0,
                       channel_multiplier=1)
        iota_f = consts.tile([P, nch_max], F32)
        nc.vector.tensor_copy(iota_f[:], iota_i[:])

        # q = query @ Wq + bq + bk   (bk folded in: tanh(q + k') with
        # k' = value@Wk needs q_total = query@Wq + bq + bk)
        q_ps = psum_pool.tile([SLOTS, D], F32, tag="kps", name="q_ps")
        for c in range(KC):
            nc.tensor.matmul(q_ps[:], QT_sb[:, c, :], Wq_sb[:, c, :],
                             start=(c == 0), stop=False)
        nc.tensor.matmul(q_ps[:], ones_row[:], bq_row[:],
                         start=False, stop=False)
        nc.tensor.matmul(q_ps[:], ones_row[:], bk_row[:],
                         start=False, stop=True)
        q_sb = consts.tile([SLOTS, D], F32)
        nc.vector.tensor_copy(q_sb[:], q_ps[:])
        # single-partition bf16 copy so each slot's q row can be a K=1
        # matmul rhs (rhs must start at partition 0)
        q_flat = consts.tile([1, SLOTS, D], BF16)
        nc.gpsimd.dma_start(q_flat[:], q_sb[:])

        len_bc = []
        mask = []
        for s in range(SLOTS):
            lb = consts.tile([P, 1], F32, tag=f"lbc{s}")
            nc.scalar.dma_start(
                lb[:], lens_f[s:s + 1, :].unsqueeze(1).to_broadcast((1, P, 1)))
            len_bc.append(lb)
            mk = consts.tile([P, nchunks[s]], F32, tag=f"mask{s}")
            nc.vector.tensor_scalar(mk[:], iota_f[:, :nchunks[s]], lb[:], None,
                                    op0=ALU.is_lt)
            mask.append(mk)

        # ---- main loop over slots, software-pipelined ----
        # The PE instruction stream is in-order, so each slot's softmax ->
        # context tail is deferred until after the NEXT slot's k-phase has
        # been emitted: the ACT/DVE softmax chain of slot s then runs under
        # slot s+1's projection matmuls instead of stalling the PE.
        g = 4

        def emit_k_phase(s):
            nch = nchunks[s]
            # slot 0 split for an early start; later slots whole-slot -
            # fewer, larger transfers keep VT supply ahead of the PE
            gv = g if s == 0 else nch
            V_sb = vsb_pool.tile([P, nch, DV], BF16, tag="vsb",
                                 name=f"vsb{s}")
            for g0 in range(0, nch, gv):
                g1 = min(g0 + gv, nch)
                src = value_d[row_off[s] + g0 * P: row_off[s] + g1 * P, :]
                nc.gpsimd.dma_start(
                    V_sb[:, g0:g1, :], src.rearrange("(c p) d -> p c d", p=P))
            # transposed layout: VT[p, j, t] = value[t, j*128 + p]
            VT_sb = vt_pool.tile([P, KC, nch * P], BF16, tag="vt",
                                 name=f"vt{s}")
            for g0 in range(0, nch, gv):
                g1 = min(g0 + gv, nch)
                for j in range(KC):
                    nc.sync.dma_start(
                        VT_sb[:, j, g0 * P:g1 * P],
                        valueT_d[j * P:(j + 1) * P,
                                 row_off[s] + g0 * P:row_off[s] + g1 * P])

            scores = sm_pool.tile([P, nch], F32, tag="scores",
                                  name=f"scores{s}")
            for c in range(nch):
                k_ps = psum_pool.tile([P, D], F32, tag="kps", name=f"kps{s}_{c}")
                for j in range(KC):
                    nc.tensor.matmul(k_ps[:],
                                     VT_sb[:, j, c * P:(c + 1) * P],
                                     Wk_sb[:, j, :],
                                     start=(j == 0), stop=False)
                # q (+bq +bk) broadcast into every t row of the chunk
                nc.tensor.matmul(k_ps[:], ones128[:], q_flat[:, s, :],
                                 start=False, stop=True)
                th = th_pool.tile([P, D], BF16, tag="th", name=f"th{s}_{c}")
                nc.scalar.activation(th[:], k_ps[:], AF.Tanh)
                scrap = scrap_pool.tile([P, D], BF16, tag="scrap",
                                        name=f"scrap{s}_{c}")
                nc.vector.scalar_tensor_tensor(
                    scrap[:], th[:], 1.0, scale_bc[:],
                    op0=ALU.bypass, op1=ALU.mult,
                    accum_out=scores[:, c:c + 1])

            # softmax front half - ACT/DVE only, no PE involvement
            # (no max subtraction; |scores| <= sum|scale| ~ 20)
            ex = sm_pool.tile([P, nch], F32, tag="ex", name=f"ex{s}")
            nc.scalar.activation(ex[:], scores[:], AF.Exp)
            exm = sm_pool.tile([P, nch], F32, tag="exm", name=f"exm{s}")
            nc.vector.tensor_tensor(exm[:], ex[:], mask[s][:], op=ALU.mult)
            rs = sm_pool.tile([P, 1], F32, tag="rs", name=f"rs{s}")
            nc.vector.reduce_sum(rs[:], exm[:], axis=mybir.AxisListType.X)
            return V_sb, exm, rs

        def emit_tail(s, V_sb, exm, rs):
            nch = nchunks[s]
            # partition-reduce via PE: s_tot[0,0] = sum_p rs[p] * 1
            s_tot = cps_pool.tile([1, 1], F32, tag="kps", name=f"stot{s}")
            nc.tensor.matmul(s_tot[:], rs[:], ones_col_f[:],
                             start=True, stop=True)
            rcp = sm_pool.tile([1, 1], F32, tag="rcp", name=f"rcp{s}")
            nc.vector.reciprocal(rcp[:], s_tot[:])

            attn = sm_pool.tile([P, nch], BF16, tag="attn", name=f"attn{s}")
            nc.vector.tensor_copy(attn[:], exm[:])

            c_ps = cps_pool.tile([1, DV], F32, tag="kps", name=f"cps{s}")
            for c in range(nch):
                nc.tensor.matmul(c_ps[:], attn[:, c:c + 1], V_sb[:, c, :],
                                 start=(c == 0), stop=(c == nch - 1))
            out_row = sm_pool.tile([1, DV], F32, tag="orow", name=f"orow{s}")
            nc.vector.tensor_scalar(out_row[:], c_ps[:], rcp[:], None,
                                    op0=ALU.mult)
            nc.sync.dma_start(out_d[s:s + 1, :], out_row[:])

        pending = None
        for s in range(SLOTS):
            st = emit_k_phase(s)
            if pending is not None:
                emit_tail(pending[0], *pending[1])
            pending = (s, st)
        emit_tail(pending[0], *pending[1])

    nc.compile()
    return nc


_graph_cache = {}

# test-harness knobs (the grading path leaves these at defaults)
TRACE = False
TRACE_KWARGS = {}
LAST_RESULTS = None


def _get_graph(nchunks):
    key = tuple(nchunks)
    if key not in _graph_cache:
        _graph_cache[key] = build_graph(key)
    return _graph_cache[key]


def plan(value_lens):
    """Sort batches by length desc; rank r -> core r%8, slot r//8.
    Returns (order, nchunks): order[s*8+c] = global batch on core c slot s."""
    lens = np.asarray(value_lens, np.int64)
    order = np.argsort(-lens, kind="stable")
    nchunks = tuple(
        int(math.ceil(max(1, int(lens[order[s * N_CORES:(s + 1) * N_CORES]].max())) / P))
        for s in range(SLOTS))
    return order, nchunks


def prepare(query, value, value_lens, Wq, bq, Wk, bk, scale):
    """Plan the batch->(core,slot) assignment, build/cache the graph, and
    pack the per-core input maps.  Returns (nc, in_maps, order, nchunks)."""
    query = np.ascontiguousarray(np.asarray(query, np.float32))
    value = np.ascontiguousarray(np.asarray(value, np.float32))
    lens = np.ascontiguousarray(np.asarray(value_lens, np.int32))
    Wq = np.ascontiguousarray(np.asarray(Wq, np.float32))
    bq = np.ascontiguousarray(np.asarray(bq, np.float32))
    Wk = np.ascontiguousarray(np.asarray(Wk, np.float32))
    bk = np.ascontiguousarray(np.asarray(bk, np.float32))
    scale = np.ascontiguousarray(np.asarray(scale, np.float32))

    order, nchunks = plan(lens)
    nc = _get_graph(nchunks)

    in_maps = []
    for c in range(N_CORES):
        bidx = [int(order[s * N_CORES + c]) for s in range(SLOTS)]
        vparts = [value[bidx[s], :nchunks[s] * P, :].astype(BF16_NP)
                  for s in range(SLOTS)]
        vpack = np.concatenate(vparts, axis=0)
        vtpack = np.concatenate([v.T for v in vparts], axis=1)
        in_maps.append({
            "value": np.ascontiguousarray(vpack),
            "valueT": np.ascontiguousarray(vtpack),
            "queryT": np.ascontiguousarray(query[bidx].astype(BF16_NP).T),
            "value_lens": np.ascontiguousarray(lens[bidx]),
            "Wq": Wq.astype(BF16_NP), "bq": bq.astype(BF16_NP),
            "Wk": Wk.astype(BF16_NP), "bk": bk.astype(BF16_NP),
            "scale": scale.astype(BF16_NP),
        })
    return nc, in_maps, order, nchunks


def kernel(query, value, value_lens, Wq, bq, Wk, bk, scale):
    nc, in_maps, order, _ = prepare(query, value, value_lens,
                                    Wq, bq, Wk, bk, scale)

    res = bass_utils.run_bass_kernel_spmd(
        nc, in_maps, core_ids=list(range(N_CORES)), trace=TRACE,
        **TRACE_KWARGS)
    global LAST_RESULTS
    LAST_RESULTS = res

    out = np.zeros((B, 1, DV), np.float32)
    for c in range(N_CORES):
        o = res.results[c]["out"]
        for s in range(SLOTS):
            out[int(order[s * N_CORES + c]), 0, :] = o[s]
    return out


# revision 43
# speedup vs baseline: 1.0591x; 1.0591x over previous
"""AddAttention (Bahdanau additive attention) Trainium2 kernel.

Math (per batch b):
    q   = query @ Wq + bq                          [D]
    k_t = value[t] @ Wk + bk                       [T, D]
    s_t = sum_d scale[d] * tanh(q[d] + k_t[d])     [T]
    a   = softmax(s masked to t < value_lens[b])
    out = sum_t a_t * value[t]                     [DV]

Distribution: pure data-parallel over batch B=32 across 8 NeuronCores
(4 batches per core, params replicated, no collectives).

Sparsity: rows t >= value_lens[b] are masked out of the softmax, so they
never influence the output.  Batches are sorted by value_lens and assigned
to (core, slot) so that slot s on every core holds a batch of similar
length; the compiled graph only processes ceil(max_len_in_slot/128) row
chunks per slot.  Masking (runtime data) handles the sub-chunk boundary,
so the same SPMD graph is correct on every core.

Layout/precision staging on the host (pure data movement + the same
f32->bf16 rounding the kernel would otherwise do on-chip): each core
receives its value rows twice in bf16 - natural [t, d] (feeds the final
context matmul) and transposed [d, t] (feeds the k = value @ Wk
projection, which contracts over d and therefore needs d on the
partition axis).  This removes all on-chip transposes and casts from the
critical path; every FLOP of the model still runs on the device.

Per-core pipeline (matmul compute bf16, accumulation fp32):
  - k projection: 4 accumulating matmuls per 128-row chunk with the
    transposed value tiles as the stationary operand, plus a 5th K=1
    matmul that broadcasts q (+bq +bk, computed on-chip) into the same
    PSUM accumulation - so ScalarE can apply tanh straight from PSUM
  - VectorE does a fused (tanh * scale) + free-axis-reduce into scores
  - softmax without max-subtraction (|s| <= sum|scale| ~ 20, safe in
    fp32); mask applied multiplicatively after exp; the normalization is
    folded into one per-batch scalar applied to the context row
  - context = attn @ value as M=1 accumulating matmuls over the natural
    bf16 value tiles; partition-sums for softmax run on the PE against a
    ones column
"""

import math
from contextlib import ExitStack

import ml_dtypes
import numpy as np

import concourse.bass as bass
import concourse.bacc as bacc
import concourse.tile as tile
from concourse import mybir
from concourse import bass_utils

F32 = mybir.dt.float32
BF16 = mybir.dt.bfloat16
I32 = mybir.dt.int32
AF = mybir.ActivationFunctionType
ALU = mybir.AluOpType

N_CORES = 8
B, TV, DQ, DV, D = 32, 2048, 512, 512, 512
SLOTS = B // N_CORES  # 4 batches per core
P = 128  # partitions / t-chunk rows
KC = D // P  # 4 contraction chunks of 128

BF16_NP = ml_dtypes.bfloat16


def build_graph(nchunks):
    """Build the per-core Bass graph. nchunks[s] = number of 128-row value
    chunks processed for slot s (same on every core -> same SPMD graph)."""
    nchunks = tuple(int(c) for c in nchunks)
    assert len(nchunks) == SLOTS and all(1 <= c <= TV // P for c in nchunks)
    nch_max = max(nchunks)
    rows = [P * c for c in nchunks]
    row_off = np.cumsum([0] + rows).tolist()
    R = row_off[-1]

    nc = bacc.Bacc("TRN2", target_bir_lowering=False, debug=False,
                   enable_asserts=False)

    value_d = nc.dram_tensor("value", [R, DV], BF16, kind="ExternalInput")
    valueT_d = nc.dram_tensor("valueT", [DV, R], BF16, kind="ExternalInput")
    query_d = nc.dram_tensor("queryT", [DQ, SLOTS], BF16, kind="ExternalInput")
    lens_d = nc.dram_tensor("value_lens", [SLOTS], I32, kind="ExternalInput")
    Wq_d = nc.dram_tensor("Wq", [DQ, D], BF16, kind="ExternalInput")
    bq_d = nc.dram_tensor("bq", [D], BF16, kind="ExternalInput")
    Wk_d = nc.dram_tensor("Wk", [DV, D], BF16, kind="ExternalInput")
    bk_d = nc.dram_tensor("bk", [D], BF16, kind="ExternalInput")
    scale_d = nc.dram_tensor("scale", [D], BF16, kind="ExternalInput")
    out_d = nc.dram_tensor("out", [SLOTS, DV], F32, kind="ExternalOutput")

    with tile.TileContext(nc) as tc, ExitStack() as ctx:
        consts = ctx.enter_context(tc.tile_pool(name="consts", bufs=1))
        vsb_pool = ctx.enter_context(tc.tile_pool(name="vsb", bufs=3))
        vt_pool = ctx.enter_context(tc.tile_pool(name="vt", bufs=3))
        th_pool = ctx.enter_context(tc.tile_pool(name="th", bufs=6))
        scrap_pool = ctx.enter_context(tc.tile_pool(name="scrap", bufs=4))
        sm_pool = ctx.enter_context(tc.tile_pool(name="sm", bufs=2))
        psum_pool = ctx.enter_context(
            tc.tile_pool(name="psum", bufs=8, space=bass.MemorySpace.PSUM))
        cps_pool = psum_pool

        # ---- constants / setup ----
        # all bf16 params arrive pre-cast; load them on the ScalarE HWDGE
        # queue so they don't contend with valueT (Sync) / value (GpSimd).
        # Order: Wk + the q-path inputs first - the PE instruction stream
        # is in-order and its head needs Wk (chunk matmuls) and q_flat
        # (folded into every chunk's accumulation group).
        # query transposed on host: QT[p, c, b] = queryT[c*128 + p, b]
        QT_sb = consts.tile([P, KC, SLOTS], BF16)
        nc.gpsimd.dma_start(
            QT_sb[:], query_d.ap().rearrange("(c p) b -> p c b", p=P))
        Wq_sb = consts.tile([P, KC, D], BF16)
        nc.gpsimd.dma_start(
            Wq_sb[:], Wq_d.ap().rearrange("(c p) n -> p c n", p=P))
        bq_row = consts.tile([1, D], BF16)
        nc.gpsimd.dma_start(bq_row[:], bq_d.ap().rearrange("(a d) -> a d", a=1))
        bk_row = consts.tile([1, D], BF16)
        nc.gpsimd.dma_start(bk_row[:], bk_d.ap().rearrange("(a d) -> a d", a=1))
        Wk_sb = consts.tile([P, KC, D], BF16)
        nc.scalar.dma_start(
            Wk_sb[:], Wk_d.ap().rearrange("(c p) n -> p c n", p=P))
        scale_row = consts.tile([1, D], BF16)
        nc.scalar.dma_start(
            scale_row[:], scale_d.ap().rearrange("(a d) -> a d", a=1))
        # broadcast scale across partitions (DMA replicated read: the
        # single-partition source is re-read P times via a 0-step free dim)
        scale_bc = consts.tile([P, D], BF16)
        nc.scalar.dma_start(scale_bc[:],
                            scale_row[:].unsqueeze(1).to_broadcast((1, P, D)))

        ones_row = consts.tile([1, SLOTS], BF16)
        nc.vector.memset(ones_row[:], 1.0)
        ones128 = consts.tile([1, P], BF16)
        nc.vector.memset(ones128[:], 1.0)
        ones_col_f = consts.tile([P, 1], F32)
        nc.vector.memset(ones_col_f[:], 1.0)
        ones512 = consts.tile([1, D], BF16)
        nc.vector.memset(ones512[:], 1.0)

        # PE warm-up: ~5 us of memset-only matmuls inside the startup DMA
        # window flips the HAM clock gate to 2.4 GHz before real work lands
        for w in range(12):
            wu = psum_pool.tile([P, D], F32, tag="kps", name=f"wu{w}")
            nc.tensor.matmul(wu[:], ones128[:], ones512[:],
                             start=True, stop=True)

        # value_lens -> f32 column, then per-slot broadcast column
        lens_i = consts.tile([SLOTS, 1], I32)
        nc.sync.dma_start(lens_i[:], lens_d.ap().rearrange("(b a) -> b a", a=1))
        lens_f = consts.tile([SLOTS, 1], F32)
        nc.vector.tensor_copy(lens_f[:], lens_i[:])

        # iota over t within the slot: iota[p, c] = c*128 + p
        iota_i = consts.tile([P, nch_max], I32)
        nc.gpsimd.iota(iota_i[:], pattern=[[P, nch_max]], base=0,
                       channel_multiplier=1)
        iota_f = consts.tile([P, nch_max], F32)
        nc.vector.tensor_copy(iota_f[:], iota_i[:])

        # q = query @ Wq + bq + bk   (bk folded in: tanh(q + k') with
        # k' = value@Wk needs q_total = query@Wq + bq + bk)
        q_ps = psum_pool.tile([SLOTS, D], F32, tag="kps", name="q_ps")
        for c in range(KC):
            nc.tensor.matmul(q_ps[:], QT_sb[:, c, :], Wq_sb[:, c, :],
                             start=(c == 0), stop=False)
        nc.tensor.matmul(q_ps[:], ones_row[:], bq_row[:],
                         start=False, stop=False)
        nc.tensor.matmul(q_ps[:], ones_row[:], bk_row[:],
                         start=False, stop=True)
        q_sb = consts.tile([SLOTS, D], F32)
        nc.vector.tensor_copy(q_sb[:], q_ps[:])
        # single-partition bf16 copy so each slot's q row can be a K=1
        # matmul rhs (rhs must start at partition 0)
        q_flat = consts.tile([1, SLOTS, D], BF16)
        nc.gpsimd.dma_start(q_flat[:], q_sb[:])

        len_bc = []
        mask = []
        for s in range(SLOTS):
            lb = consts.tile([P, 1], F32, tag=f"lbc{s}")
            nc.scalar.dma_start(
                lb[:], lens_f[s:s + 1, :].unsqueeze(1).to_broadcast((1, P, 1)))
            len_bc.append(lb)
            mk = consts.tile([P, nchunks[s]], F32, tag=f"mask{s}")
            nc.vector.tensor_scalar(mk[:], iota_f[:, :nchunks[s]], lb[:], None,
                                    op0=ALU.is_lt)
            mask.append(mk)

        # ---- main loop over slots, software-pipelined ----
        # The PE instruction stream is in-order, so each slot's softmax ->
        # context tail is deferred until after the NEXT slot's k-phase has
        # been emitted: the ACT/DVE softmax chain of slot s then runs under
        # slot s+1's projection matmuls instead of stalling the PE.
        g = 4

        def emit_k_phase(s):
            nch = nchunks[s]
            gv = g
            V_sb = vsb_pool.tile([P, nch, DV], BF16, tag="vsb",
                                 name=f"vsb{s}")
            for g0 in range(0, nch, gv):
                g1 = min(g0 + gv, nch)
                src = value_d[row_off[s] + g0 * P: row_off[s] + g1 * P, :]
                nc.gpsimd.dma_start(
                    V_sb[:, g0:g1, :], src.rearrange("(c p) d -> p c d", p=P))
            # transposed layout: VT[p, j, t] = value[t, j*128 + p]
            VT_sb = vt_pool.tile([P, KC, nch * P], BF16, tag="vt",
                                 name=f"vt{s}")
            for g0 in range(0, nch, gv):
                g1 = min(g0 + gv, nch)
                for j in range(KC):
                    nc.sync.dma_start(
                        VT_sb[:, j, g0 * P:g1 * P],
                        valueT_d[j * P:(j + 1) * P,
                                 row_off[s] + g0 * P:row_off[s] + g1 * P])

            scores = sm_pool.tile([P, nch], F32, tag="scores",
                                  name=f"scores{s}")
            for c in range(nch):
                k_ps = psum_pool.tile([P, D], F32, tag="kps", name=f"kps{s}_{c}")
                for j in range(KC):
                    nc.tensor.matmul(k_ps[:],
                                     VT_sb[:, j, c * P:(c + 1) * P],
                                     Wk_sb[:, j, :],
                                     start=(j == 0), stop=False)
                # q (+bq +bk) broadcast into every t row of the chunk
                nc.tensor.matmul(k_ps[:], ones128[:], q_flat[:, s, :],
                                 start=False, stop=True)
                th = th_pool.tile([P, D], BF16, tag="th", name=f"th{s}_{c}")
                nc.scalar.activation(th[:], k_ps[:], AF.Tanh)
                scrap = scrap_pool.tile([P, D], BF16, tag="scrap",
                                        name=f"scrap{s}_{c}")
                nc.vector.scalar_tensor_tensor(
                    scrap[:], th[:], 1.0, scale_bc[:],
                    op0=ALU.bypass, op1=ALU.mult,
                    accum_out=scores[:, c:c + 1])

            # softmax front half - ACT/DVE only, no PE involvement
            # (no max subtraction; |scores| <= sum|scale| ~ 20)
            ex = sm_pool.tile([P, nch], F32, tag="ex", name=f"ex{s}")
            nc.scalar.activation(ex[:], scores[:], AF.Exp)
            exm = sm_pool.tile([P, nch], F32, tag="exm", name=f"exm{s}")
            nc.vector.tensor_tensor(exm[:], ex[:], mask[s][:], op=ALU.mult)
            rs = sm_pool.tile([P, 1], F32, tag="rs", name=f"rs{s}")
            nc.vector.reduce_sum(rs[:], exm[:], axis=mybir.AxisListType.X)
            return V_sb, exm, rs

        def emit_tail(s, V_sb, exm, rs):
            nch = nchunks[s]
            # partition-reduce via PE: s_tot[0,0] = sum_p rs[p] * 1
            s_tot = cps_pool.tile([1, 1], F32, tag="kps", name=f"stot{s}")
            nc.tensor.matmul(s_tot[:], rs[:], ones_col_f[:],
                             start=True, stop=True)
            rcp = sm_pool.tile([1, 1], F32, tag="rcp", name=f"rcp{s}")
            nc.vector.reciprocal(rcp[:], s_tot[:])

            attn = sm_pool.tile([P, nch], BF16, tag="attn", name=f"attn{s}")
            nc.vector.tensor_copy(attn[:], exm[:])

            c_ps = cps_pool.tile([1, DV], F32, tag="kps", name=f"cps{s}")
            for c in range(nch):
                nc.tensor.matmul(c_ps[:], attn[:, c:c + 1], V_sb[:, c, :],
                                 start=(c == 0), stop=(c == nch - 1))
            out_row = sm_pool.tile([1, DV], F32, tag="orow", name=f"orow{s}")
            nc.vector.tensor_scalar(out_row[:], c_ps[:], rcp[:], None,
                                    op0=ALU.mult)
            nc.sync.dma_start(out_d[s:s + 1, :], out_row[:])

        pending = None
        for s in range(SLOTS):
            st = emit_k_phase(s)
            if pending is not None:
                emit_tail(pending[0], *pending[1])
            pending = (s, st)
        emit_tail(pending[0], *pending[1])

    nc.compile()
    return nc


_graph_cache = {}

# test-harness knobs (the grading path leaves these at defaults)
TRACE = False
TRACE_KWARGS = {}
LAST_RESULTS = None


def _get_graph(nchunks):
    key = tuple(nchunks)
    if key not in _graph_cache:
        _graph_cache[key] = build_graph(key)
    return _graph_cache[key]


def plan(value_lens):
    """Sort batches by length desc; rank r -> core r%8, slot r//8.
    Returns (order, nchunks): order[s*8+c] = global batch on core c slot s."""
    lens = np.asarray(value_lens, np.int64)
    order = np.argsort(-lens, kind="stable")
    nchunks = tuple(
        int(math.ceil(max(1, int(lens[order[s * N_CORES:(s + 1) * N_CORES]].max())) / P))
        for s in range(SLOTS))
    return order, nchunks


def prepare(query, value, value_lens, Wq, bq, Wk, bk, scale):
    """Plan the batch->(core,slot) assignment, build/cache the graph, and
    pack the per-core input maps.  Returns (nc, in_maps, order, nchunks)."""
    query = np.ascontiguousarray(np.asarray(query, np.float32))
    value = np.ascontiguousarray(np.asarray(value, np.float32))
    lens = np.ascontiguousarray(np.asarray(value_lens, np.int32))
    Wq = np.ascontiguousarray(np.asarray(Wq, np.float32))
    bq = np.ascontiguousarray(np.asarray(bq, np.float32))
    Wk = np.ascontiguousarray(np.asarray(Wk, np.float32))
    bk = np.ascontiguousarray(np.asarray(bk, np.float32))
    scale = np.ascontiguousarray(np.asarray(scale, np.float32))

    order, nchunks = plan(lens)
    nc = _get_graph(nchunks)

    in_maps = []
    for c in range(N_CORES):
        bidx = [int(order[s * N_CORES + c]) for s in range(SLOTS)]
        vparts = [value[bidx[s], :nchunks[s] * P, :].astype(BF16_NP)
                  for s in range(SLOTS)]
        vpack = np.concatenate(vparts, axis=0)
        vtpack = np.concatenate([v.T for v in vparts], axis=1)
        in_maps.append({
            "value": np.ascontiguousarray(vpack),
            "valueT": np.ascontiguousarray(vtpack),
            "queryT": np.ascontiguousarray(query[bidx].astype(BF16_NP).T),
            "value_lens": np.ascontiguousarray(lens[bidx]),
            "Wq": Wq.astype(BF16_NP), "bq": bq.astype(BF16_NP),
            "Wk": Wk.astype(BF16_NP), "bk": bk.astype(BF16_NP),
            "scale": scale.astype(BF16_NP),
        })
    return nc, in_maps, order, nchunks


def kernel(query, value, value_lens, Wq, bq, Wk, bk, scale):
    nc, in_maps, order, _ = prepare(query, value, value_lens,
                                    Wq, bq, Wk, bk, scale)

    res = bass_utils.run_bass_kernel_spmd(
        nc, in_maps, core_ids=list(range(N_CORES)), trace=TRACE,
        **TRACE_KWARGS)
    global LAST_RESULTS
    LAST_RESULTS = res

    out = np.zeros((B, 1, DV), np.float32)
    for c in range(N_CORES):
        o = res.results[c]["out"]
        for s in range(SLOTS):
            out[int(order[s * N_CORES + c]), 0, :] = o[s]
    return out
